# revision 2
# baseline (speedup 1.0000x reference)
"""Trainium2 Bass kernel for nn_Encoder segment-reduce.

Reference computation (per sample b):
    cls = onehot(argmax_k outputs[b])            # [K, HW]
    sizes = cls.sum(HW) + 0.01                   # [K]
    feat_set = feats[b] @ cls.T / sizes          # [F, K]
    out[b] = w_proj @ feat_set + bias            # [E, K]

Kernel strategy (pure data parallel: 1 sample per NeuronCore, 8 cores).

Segment-reduce FIRST (the cheap contraction), projection second:
    feat_setT[k, f] = sum_hw onehot[hw, k] * featsT[hw, f]
computed with the onehot chunk [128hw, 21] as the PE's stationary operand and
featsT chunks [128hw, 512f] as the moving operand, accumulating four [21, 512]
PSUM tiles across all 32 hw chunks.  This streams feats through the PE exactly
once (65K cycles) — the minimum possible.

feats dtype is fp8 e3m4 (TRN FP8_EXP3): N(0,1) data fits the e3m4 range
(max normal 15.5) and its 4 mantissa bits give rel err ~1.2e-2 end to end
(measured vs the f32 reference; threshold 2e-2).  This halves HBM traffic vs
bf16: 8.39 MB feats + 1 MB wT (bf16) + 0.34 MB outputs (f32) ≈ 9.75 MB/core
≈ 27.2 us at the 358 GB/s per-core HBM limit, balanced against the 65536-cycle
(27.3 us) PE stream.  The onehot matmuls run in normal (single-rate) fp8 mode:
double-fp8 would upcast operands to e6m3 and destroy e3m4's 4th mantissa bit
(measured 3.0e-2 rel err — fails).

DMA order: outputs (f32) first so the DVE argmax + PE size-count matmuls fill
the initial feats-DMA window, then the 8x1MB feats blocks, then wT/bias (only
needed by the tail, which starts after the stream ends).

After the stream: scale rows by 1/sizes, PE-transpose the [21, 2048] result
back to f-major in 128-col chunks, and apply the (tiny) w_proj projection +
bias, writing [E, K] directly.

A burst of dummy matmuls at kernel start keeps the PE's HAM clock gate warm
through the initial DMA window (cold PE runs at 1.2 GHz vs 2.4 GHz warm).

dtype: "fp8" (e3m4 feats, rel err ~1.2e-2), "bf16" (rel err ~3e-3) or "f32r".
"""

import numpy as np

import concourse.bacc as bacc
import concourse.bass as bass
import concourse.mybir as mybir
import concourse.tile as tile
from concourse.bass import ds, ts
from concourse.bass_utils import run_bass_kernel_spmd
from concourse.masks import make_identity

# Problem shapes (hardcoded per contract)
B = 8
K = 21
H = 64
W = 64
HW = H * W            # 4096
F = 2048
E = 256
P = 128
FC = F // P           # 16 f-chunks of 128
FG = 4                # f-groups of 512 (psum accumulate tiles)
FGW = F // FG         # 512
N_T = HW // P         # 32 hw chunks
N_CORES = 8

F32 = mybir.dt.float32
F32R = mybir.dt.float32r
BF16 = mybir.dt.bfloat16
FP8 = mybir.dt.float8e3   # e3m4: 4 mantissa bits

DTYPE = "fp8"         # "fp8", "bf16" or "f32r"


def _mm_dt(dtype):
    return {"fp8": FP8, "bf16": BF16, "f32r": F32R}[dtype]


def build_module(dtype=DTYPE, warmup=100, tb=None):
    mm_dt = _mm_dt(dtype)
    # dtype of the (tiny) projection tail: f32r producers are awkward for
    # the tail ops, so the f32r path runs its tail in plain fp32.
    pj_dt = F32 if dtype == "f32r" else BF16
    # hw chunks per DMA block: 8KB contiguous per partition per block.
    if tb is None:
        tb = 4 if dtype == "fp8" else 2
    n_blk = N_T // tb
    nc = bacc.Bacc("TRN2", target_bir_lowering=False, debug=False)

    # outputs host-transposed to [p, t, k] (pixel-major).
    outputs_d = nc.dram_tensor("outputs_in", [P, N_T, K], F32, kind="ExternalInput")
    # featsT host-permuted to [p, t, fgrp, fj]: featsT[t*128+p, fgrp*512+fj].
    feats_d = nc.dram_tensor(
        "feats_in", [P, N_T, FG, FGW], mm_dt, kind="ExternalInput"
    )
    wT_d = nc.dram_tensor("wT_in", [F, E], pj_dt, kind="ExternalInput")
    bias_d = nc.dram_tensor("bias_in", [E], F32, kind="ExternalInput")
    out_d = nc.dram_tensor("out", [E, K], F32, kind="ExternalOutput")

    with tile.TileContext(nc) as tc:
        with (
            tc.tile_pool(name="consts", bufs=1) as consts,
            tc.tile_pool(name="feats", bufs=n_blk) as feats_pool,
            tc.tile_pool(name="small", bufs=4) as small,
            tc.tile_pool(name="outp", bufs=1) as outp,
            tc.tile_pool(name="ps_fs", bufs=1, space="PSUM") as ps_fs,
            tc.tile_pool(name="ps_sz", bufs=1, space="PSUM") as ps_sz,
            tc.tile_pool(name="ps_misc", bufs=3, space="PSUM") as ps_misc,
        ):
            # Bulk DMAs in FIFO order on the sync HWDGE queue: outputs first
            # (argmax + size counts fill the feats wait), then the feats
            # stream, then wT/bias (needed only by the tail).
            outputs_sb = consts.tile([P, N_T, K], F32)
            nc.sync.dma_start(out=outputs_sb, in_=outputs_d.ap())
            feats_r = feats_d.ap()
            fgs = []
            for g in range(n_blk):
                fg = feats_pool.tile([P, tb, FG, FGW], mm_dt, name=f"fg{g}",
                                     tag="fg")
                nc.sync.dma_start(out=fg, in_=feats_r[:, ds(g * tb, tb)])
                fgs.append(fg)
            wT_sb = consts.tile([P, FC, E], pj_dt)
            nc.sync.dma_start(
                out=wT_sb, in_=wT_d.ap().rearrange("(fc p) e -> p fc e", p=P)
            )
            bias_sb = consts.tile([P, 2], F32)
            nc.sync.dma_start(
                out=bias_sb, in_=bias_d.ap().rearrange("(ec p) -> p ec", p=P)
            )

            # PE warm-up: HAM holds the PE at 1.2 GHz until ~3.4us of
            # sustained activity; dummy matmuls bridge the initial DMA wait.
            warm_w = consts.tile([P, 64], BF16)
            nc.vector.memset(warm_w, 0.0)
            warm_ps = ps_misc.tile([P, 64], F32, tag="m")
            for _ in range(warmup):
                nc.tensor.matmul(warm_ps[0:64, :], lhsT=warm_w, rhs=warm_w)

            ident = consts.tile([P, P], F32)
            make_identity(nc, ident)
            ones_b = consts.tile([P, 2], mm_dt)
            if dtype == "f32r":
                ones_f = consts.tile([P, 2], F32)
                nc.vector.memset(ones_f, 1.0)
                nc.vector.tensor_copy(ones_b, ones_f)
            else:
                nc.vector.memset(ones_b, 1.0)

            # Phase 1 (DVE only): onehot = (outT == rowmax) per hw chunk.
            oh_all = consts.tile([P, N_T, K], mm_dt)
            for t in range(N_T):
                rowmax = small.tile([P, 1], F32)
                nc.vector.tensor_reduce(
                    rowmax, outputs_sb[:, t, :], mybir.AxisListType.X,
                    mybir.AluOpType.max,
                )
                nc.vector.tensor_scalar(
                    out=oh_all[:, t, :],
                    in0=outputs_sb[:, t, :],
                    scalar1=rowmax,
                    scalar2=None,
                    op0=mybir.AluOpType.is_equal,
                )

            # Class sizes: onehot.T @ ones accumulated over all 32 chunks.
            # Pure PE work on oh_all — runs while the feats stream arrives.
            sz_ps = ps_sz.tile([K, 2], F32)
            for t in range(N_T):
                nc.tensor.matmul(
                    sz_ps,
                    lhsT=oh_all[:, t, :],
                    rhs=ones_b,
                    start=(t == 0),
                    stop=(t == N_T - 1),
                )
            sizes_sb = small.tile([K, 1], F32, tag="sizes")
            nc.vector.tensor_scalar_add(sizes_sb, sz_ps[:, 0:1], 0.01)
            recip = small.tile([K, 1], F32, tag="recip")
            nc.vector.reciprocal(recip, sizes_sb)

            # Segment-reduce stream: feat_setT[k, f] accumulates in PSUM
            # across all 32 hw chunks; feats passes the PE exactly once.
            fs_ps = [
                ps_fs.tile([K, FGW], F32, name=f"fs{i}", tag=f"fs{i}")
                for i in range(FG)
            ]
            for g in range(n_blk):
                fg = fgs[g]
                for ti in range(tb):
                    t = g * tb + ti
                    oh_t = oh_all[:, t, :]
                    for fgrp in range(FG):
                        nc.tensor.matmul(
                            fs_ps[fgrp],
                            lhsT=oh_t,
                            rhs=fg[:, ti, fgrp, :],
                            start=(t == 0),
                            stop=(t == N_T - 1),
                        )

            # Tail: divide by sizes (fused into the PSUM->SBUF copies, split
            # across DVE and ACT), transpose feat_set back to f-major,
            # project with w_proj, add bias, store [E, K].
            fs_sc = consts.tile([K, F], pj_dt)
            for fgrp in range(FG):
                if fgrp % 2 == 0:
                    nc.vector.tensor_scalar_mul(
                        fs_sc[:, ds(fgrp * FGW, FGW)], fs_ps[fgrp], recip
                    )
                else:
                    nc.scalar.activation(
                        out=fs_sc[:, ds(fgrp * FGW, FGW)],
                        in_=fs_ps[fgrp],
                        func=mybir.ActivationFunctionType.Copy,
                        scale=recip,
                    )

            ident_b = consts.tile([K, K], pj_dt)
            nc.vector.tensor_copy(ident_b, ident[:K, :K])
            fsT_sb = consts.tile([P, FC, K], pj_dt)
            ps_o = [None, None]
            out_sb = outp.tile([P, 2, K], F32)
            for ec in range(2):
                ps_o_ec = ps_misc.tile([P, K], F32, tag="m", name=f"ps_o{ec}")
                ps_o[ec] = ps_o_ec
            for fc in range(FC):
                # trp reuses the ps_fs slots (free once the scales are done),
                # giving the transpose->copy chain a 4-deep pipeline.
                trp = ps_fs.tile(
                    [P, K], pj_dt, name=f"trp{fc}", tag=f"fs{fc % FG}"
                )
                nc.tensor.transpose(trp, fs_sc[:, ts(fc, P)], ident_b)
                nc.vector.tensor_copy(fsT_sb[:, fc, :], trp)
                for ec in range(2):
                    nc.tensor.matmul(
                        ps_o[ec],
                        lhsT=wT_sb[:, fc, ds(ec * P, P)],
                        rhs=fsT_sb[:, fc, :],
                        start=(fc == 0),
                        stop=(fc == FC - 1),
                    )
            for ec in range(2):
                nc.vector.tensor_scalar_add(
                    out_sb[:, ec, :], ps_o[ec], bias_sb[:, ec : ec + 1]
                )
            nc.sync.dma_start(
                out=out_d.ap().rearrange("(ec p) k -> p ec k", p=P), in_=out_sb
            )

    nc.compile()
    return nc


_CACHE = {}


def make_in_maps(outputs, feats, w_proj, b_proj, dtype=DTYPE):
    import ml_dtypes

    mm_np = {
        "fp8": ml_dtypes.float8_e3m4,
        "bf16": ml_dtypes.bfloat16,
        "f32r": np.float32,
    }[dtype]
    pj_np = np.float32 if dtype == "f32r" else ml_dtypes.bfloat16
    outputs = np.asarray(outputs, dtype=np.float32)
    # [B, K, H, W] -> per sample [p, t, k] (pixel-major: hw = t*128 + p)
    outputs_t = np.ascontiguousarray(
        outputs.reshape(B, K, N_T, P).transpose(0, 3, 2, 1)
    )
    feats = np.asarray(feats, dtype=np.float32).astype(mm_np)
    # [B, F, H, W] -> per sample [p, t, fgrp, fj] = featsT[t*128+p, fgrp*512+fj]
    feats_sh = np.ascontiguousarray(
        feats.reshape(B, FG, FGW, N_T, P).transpose(0, 4, 3, 1, 2)
    )
    wT = np.ascontiguousarray(np.asarray(w_proj, dtype=np.float32).T.astype(pj_np))
    bias = np.ascontiguousarray(np.asarray(b_proj, dtype=np.float32))
    return [
        {
            "outputs_in": outputs_t[b],
            "feats_in": feats_sh[b],
            "wT_in": wT,
            "bias_in": bias,
        }
        for b in range(B)
    ]


def kernel(outputs, feats, w_proj, b_proj, _trace=False, _trace_kwargs=None,
           _dtype=DTYPE, _build_kwargs=None):
    key = (_dtype, tuple(sorted((_build_kwargs or {}).items())))
    if key not in _CACHE:
        _CACHE[key] = build_module(dtype=_dtype, **(_build_kwargs or {}))
    nc = _CACHE[key]
    in_maps = make_in_maps(outputs, feats, w_proj, b_proj, dtype=_dtype)
    res = run_bass_kernel_spmd(
        nc,
        in_maps,
        core_ids=list(range(N_CORES)),
        trace=_trace,
        **(_trace_kwargs or {}),
    )
    out = np.stack([np.asarray(r["out"]) for r in res.results])
    if _trace:
        _CACHE["last_results"] = res
    return out


# revision 6
# speedup vs baseline: 1.1440x; 1.1440x over previous
"""Trainium2 Bass kernel for nn_Encoder segment-reduce.

Reference computation (per sample b):
    cls = onehot(argmax_k outputs[b])            # [K, HW]
    sizes = cls.sum(HW) + 0.01                   # [K]
    feat_set = feats[b] @ cls.T / sizes          # [F, K]
    out[b] = w_proj @ feat_set + bias            # [E, K]

Kernel strategy (pure data parallel: 1 sample per NeuronCore, 8 cores).

Segment-reduce FIRST (the cheap contraction), projection second:
    feat_setT[k, f] = sum_hw onehot[hw, k] * featsT[hw, f]
with the onehot chunk [128hw, 21] as the PE's stationary operand and featsT
chunks [128hw, 512f] as the moving operand.  The four f-group matmuls of each
hw chunk are packed into the four 32-column groups of the PE array via
tile_position=(0, 32j): the stationary onehot only occupies 21 of 128 array
columns, so the four matmuls execute concurrently and the whole stream costs
~2x the DMA-free-chunk time instead of 4x.  One [128, 512] PSUM tile holds all
four accumulators (f-group j at partitions 32j..32j+21).

feats dtype is fp8 e3m4 (TRN FP8_EXP3): N(0,1) data fits the e3m4 range and
its 4 mantissa bits give rel err ~1.2e-2 end to end (threshold 2e-2), halving
HBM traffic vs bf16: 8.39 MB feats + 1 MB wT (bf16) + 0.34 MB outputs (f32)
~= 9.75 MB/core ~= 27 us at the per-core HBM limit -- the kernel is DMA-bound.
The matmuls run in normal (single-rate) fp8 mode: double-fp8 would upcast
operands to e6m3 and destroy e3m4's 4th mantissa bit (measured 3.0e-2 -- fails).

DMA order: outputs (f32) + bias first so the DVE argmax fills the initial
feats window, then the feats blocks (the last block split into single-chunk
pieces so the stream tail is not gated on a full 1MB transfer), then wT.
The per-chunk size-count matmul is interleaved with the stream so the PE
never serializes behind the (DVE-paced) argmax.

Tail: replicate 1/sizes to all four column groups with a tiny PE matmul,
scale the PSUM accumulators into bf16, PE-transpose the [21, 2048] result
back to f-major using a replicated identity (each column group transposes
from its own 32-row block), then project against wT with feat_set chunks as
the stationary operand ([1, 21]-cheap weight loads, 256-wide moving wT) into
a [21, 256] PSUM tile pre-initialized with the bias via an outer-product
matmul.  The output is stored as [K, E]; the host transposes when gathering.

A burst of dummy matmuls at kernel start keeps the PE's HAM clock gate warm
through the initial DMA window (cold PE runs at 1.2 GHz vs 2.4 GHz warm).

dtype: "fp8" (e3m4 feats, rel err ~1.2e-2) or "bf16" (rel err ~3e-3).
"""

import numpy as np

import concourse.bacc as bacc
import concourse.bass as bass
import concourse.mybir as mybir
import concourse.tile as tile
from concourse.bass import ds, ts
from concourse.bass_utils import run_bass_kernel_spmd
from concourse.masks import make_identity

# Problem shapes (hardcoded per contract)
B = 8
K = 21
H = 64
W = 64
HW = H * W            # 4096
F = 2048
E = 256
P = 128
FC = F // P           # 16 f-chunks of 128
FG = 4                # f-groups of 512 (PE column groups)
FGW = F // FG         # 512
N_T = HW // P         # 32 hw chunks
N_CORES = 8

F32 = mybir.dt.float32
BF16 = mybir.dt.bfloat16
FP8 = mybir.dt.float8e3   # e3m4: 4 mantissa bits

DTYPE = "fp8"         # "fp8" or "bf16"


def build_module(dtype=DTYPE, warmup=100):
    mm_dt = FP8 if dtype == "fp8" else BF16
    nc = bacc.Bacc("TRN2", target_bir_lowering=False, debug=False)

    # outputs host-transposed to [p, t, k] (pixel-major).
    outputs_d = nc.dram_tensor("outputs_in", [P, N_T, K], F32, kind="ExternalInput")
    # featsT host-permuted to [p, t, fgrp, fj]: featsT[t*128+p, fgrp*512+fj].
    feats_d = nc.dram_tensor(
        "feats_in", [P, N_T, FG, FGW], mm_dt, kind="ExternalInput"
    )
    wT_d = nc.dram_tensor("wT_in", [F, E], BF16, kind="ExternalInput")
    bias_d = nc.dram_tensor("bias_in", [E], F32, kind="ExternalInput")
    # out.T -- the host transposes each sample's [K, E] result when gathering.
    out_d = nc.dram_tensor("out", [K, E], F32, kind="ExternalOutput")

    # feats DMA blocks (in hw chunks): 1MB blocks, last block split into
    # single-chunk pieces so the stream tail sees data ASAP.
    blocks = [(g * 4, 4) for g in range(7)] + [(28 + i, 1) for i in range(4)]

    with tile.TileContext(nc) as tc:
        with (
            tc.tile_pool(name="consts", bufs=1) as consts,
            tc.tile_pool(name="feats", bufs=len(blocks)) as feats_pool,
            tc.tile_pool(name="small", bufs=4) as small,
            tc.tile_pool(name="outp", bufs=1) as outp,
            tc.tile_pool(name="ps_fs", bufs=1, space="PSUM") as ps_fs,
            tc.tile_pool(name="ps_sz", bufs=1, space="PSUM") as ps_sz,
            tc.tile_pool(name="ps_tr", bufs=1, space="PSUM") as ps_tr,
            tc.tile_pool(name="ps_misc", bufs=2, space="PSUM") as ps_misc,
        ):
            # Bulk DMAs in FIFO order on the sync HWDGE queue.
            outputs_sb = consts.tile([P, N_T, K], F32)
            nc.sync.dma_start(out=outputs_sb, in_=outputs_d.ap())
            bias_sb = consts.tile([1, E], F32)
            nc.sync.dma_start(
                out=bias_sb, in_=bias_d.ap().rearrange("(o e) -> o e", o=1)
            )
            feats_r = feats_d.ap()
            fgs = []
            for bi, (t0, tb) in enumerate(blocks):
                fg = feats_pool.tile([P, tb, FG, FGW], mm_dt, name=f"fg{bi}",
                                     tag="fg")
                nc.sync.dma_start(out=fg, in_=feats_r[:, ds(t0, tb)])
                fgs.append(fg)
            wT_sb = consts.tile([P, FC, E], BF16)
            nc.sync.dma_start(
                out=wT_sb, in_=wT_d.ap().rearrange("(fc p) e -> p fc e", p=P)
            )

            # PE warm-up: HAM holds the PE at 1.2 GHz until ~3.4us of
            # sustained activity; dummy matmuls bridge the initial DMA wait.
            warm_w = consts.tile([P, 64], BF16)
            nc.vector.memset(warm_w, 0.0)
            warm_ps = ps_misc.tile([P, 64], F32, tag="m")
            for _ in range(warmup):
                nc.tensor.matmul(warm_ps[0:64, :], lhsT=warm_w, rhs=warm_w)

            # Constants.  rep_sb[k, 32j+k'] = delta(k,k') replicates a [21]
            # partition vector to all four 32-row column-group offsets; it is
            # built with free-dim-shifted copies of the identity (partition
            # shifts are impossible on DVE, free shifts are not).
            ident = consts.tile([P, P], F32)
            make_identity(nc, ident)
            rep_sb = consts.tile([K, P], F32)
            nc.vector.memset(rep_sb, 0.0)
            for j in range(FG):
                nc.vector.tensor_copy(rep_sb[:, ds(32 * j, K)], ident[:K, :K])
            # ident_rep[32j+k, k'] = delta(k,k'): per-column-group identity
            # for the tail transposes, built on the PE (rep_sb.T @ I21).
            identrep_ps = ps_tr.tile([P, K], F32, tag="t0")
            nc.tensor.matmul(
                identrep_ps, lhsT=rep_sb, rhs=ident[:K, :K],
                start=True, stop=True,
            )
            ident_rep = consts.tile([P, K], BF16)
            nc.vector.tensor_copy(ident_rep, identrep_ps)
            ones_b = consts.tile([P, 2], mm_dt)
            nc.vector.memset(ones_b, 1.0)
            ones21_b = consts.tile([1, K], BF16)
            nc.vector.memset(ones21_b, 1.0)
            bias_b = consts.tile([1, E], BF16)
            nc.vector.tensor_copy(bias_b, bias_sb)
            # Force the ACT engine's table load off the critical path.
            dummy_act = small.tile([1, 2], F32, tag="da")
            nc.scalar.activation(
                out=dummy_act, in_=bias_sb[:, 0:2],
                func=mybir.ActivationFunctionType.Copy,
            )

            # Phase 1 (DVE only): onehot = (outT == rowmax) per hw chunk.
            oh_all = consts.tile([P, N_T, K], mm_dt)
            for t in range(N_T):
                rowmax = small.tile([P, 1], F32)
                nc.vector.tensor_reduce(
                    rowmax, outputs_sb[:, t, :], mybir.AxisListType.X,
                    mybir.AluOpType.max,
                )
                nc.vector.tensor_scalar(
                    out=oh_all[:, t, :],
                    in0=outputs_sb[:, t, :],
                    scalar1=rowmax,
                    scalar2=None,
                    op0=mybir.AluOpType.is_equal,
                )

            # Segment-reduce stream.  Per hw chunk: one size-count matmul
            # (onehot.T @ ones -> [21, 2]) plus four f-group matmuls packed
            # into the four PE column groups, accumulating [128, 512] PSUM
            # (f-group j at partitions 32j..32j+21) across all 32 chunks.
            fs_ps = ps_fs.tile([P, FGW], F32)
            sz_ps = ps_sz.tile([K, 2], F32, tag="sz")
            for bi, (t0, tb) in enumerate(blocks):
                fg = fgs[bi]
                for ti in range(tb):
                    t = t0 + ti
                    oh_t = oh_all[:, t, :]
                    nc.tensor.matmul(
                        sz_ps,
                        lhsT=oh_t,
                        rhs=ones_b,
                        start=(t == 0),
                        stop=(t == N_T - 1),
                    )
                    for j in range(FG):
                        nc.tensor.matmul(
                            fs_ps[ds(32 * j, K), :],
                            lhsT=oh_t,
                            rhs=fg[:, ti, j, :],
                            start=(t == 0),
                            stop=(t == N_T - 1),
                            tile_position=(0, 32 * j),
                        )

            # 1/sizes, replicated to all four column groups on the PE.
            sizes_sb = small.tile([K, 1], F32, tag="sizes")
            nc.vector.tensor_scalar_add(sizes_sb, sz_ps[:, 0:1], 0.01)
            recip = small.tile([K, 1], F32, tag="recip")
            nc.vector.reciprocal(recip, sizes_sb)
            recip_ps = ps_sz.tile([P, 1], F32, name="recip128", tag="sz")
            nc.tensor.matmul(recip_ps, lhsT=rep_sb, rhs=recip,
                             start=True, stop=True)
            recip128 = small.tile([P, 1], F32, tag="r128")
            nc.vector.tensor_copy(recip128, recip_ps)

            # Scale by 1/sizes during the PSUM->SBUF copy (DVE + ACT halves).
            fs_sc = consts.tile([P, FGW], BF16)
            nc.vector.tensor_scalar_mul(
                fs_sc[:, 0 : FGW // 2], fs_ps[:, 0 : FGW // 2], recip128
            )
            nc.scalar.activation(
                out=fs_sc[:, ds(FGW // 2, FGW // 2)],
                in_=fs_ps[:, ds(FGW // 2, FGW // 2)],
                func=mybir.ActivationFunctionType.Copy,
                scale=recip128,
            )

            # Tail: transpose feat_set chunks back to f-major (each column
            # group transposes out of its own 32-row block), then project:
            # out.T[k, e] = sum_f fs[k, f] wT[f, e], with the [128f, 21]
            # chunks stationary and wT moving.  The PSUM accumulator is
            # pre-initialized with the bias via a [1,21]x[1,256] outer
            # product, so no separate bias add is needed.
            fsT_sb = consts.tile([P, FC, K], BF16)
            proj_ps = ps_misc.tile([K, E], F32, tag="m", name="proj")
            nc.tensor.matmul(proj_ps, lhsT=ones21_b, rhs=bias_b,
                             start=True, stop=False, skip_group_check=True)
            for fc in range(FC):
                j, c = fc // FG, fc % FG
                trp = ps_tr.tile([P, K], BF16, name=f"trp{fc}",
                                 tag=f"t{fc % 3}")
                nc.tensor.transpose(
                    trp,
                    fs_sc[ds(32 * j, K), ts(c, P)],
                    ident_rep[ds(32 * j, K), :],
                    tile_position=(32 * j, 0),
                )
                if fc % 2 == 0:
                    nc.vector.tensor_copy(fsT_sb[:, fc, :], trp)
                else:
                    nc.scalar.activation(
                        out=fsT_sb[:, fc, :], in_=trp,
                        func=mybir.ActivationFunctionType.Copy,
                    )
                nc.tensor.matmul(
                    proj_ps,
                    lhsT=fsT_sb[:, fc, :],
                    rhs=wT_sb[:, fc, :],
                    start=False,
                    stop=(fc == FC - 1),
                    skip_group_check=True,
                )
            out_sb = outp.tile([K, E], F32)
            nc.vector.tensor_copy(out_sb, proj_ps)
            nc.sync.dma_start(out=out_d.ap(), in_=out_sb)

    nc.compile()
    return nc


_CACHE = {}


def make_in_maps(outputs, feats, w_proj, b_proj, dtype=DTYPE):
    import ml_dtypes

    mm_np = ml_dtypes.float8_e3m4 if dtype == "fp8" else ml_dtypes.bfloat16
    outputs = np.asarray(outputs, dtype=np.float32)
    # [B, K, H, W] -> per sample [p, t, k] (pixel-major: hw = t*128 + p)
    outputs_t = np.ascontiguousarray(
        outputs.reshape(B, K, N_T, P).transpose(0, 3, 2, 1)
    )
    feats = np.asarray(feats, dtype=np.float32).astype(mm_np)
    # [B, F, H, W] -> per sample [p, t, fgrp, fj] = featsT[t*128+p, fgrp*512+fj]
    feats_sh = np.ascontiguousarray(
        feats.reshape(B, FG, FGW, N_T, P).transpose(0, 4, 3, 1, 2)
    )
    wT = np.ascontiguousarray(
        np.asarray(w_proj, dtype=np.float32).T.astype(ml_dtypes.bfloat16)
    )
    bias = np.ascontiguousarray(np.asarray(b_proj, dtype=np.float32))
    return [
        {
            "outputs_in": outputs_t[b],
            "feats_in": feats_sh[b],
            "wT_in": wT,
            "bias_in": bias,
        }
        for b in range(B)
    ]


def kernel(outputs, feats, w_proj, b_proj, _trace=False, _trace_kwargs=None,
           _dtype=DTYPE, _build_kwargs=None):
    key = (_dtype, tuple(sorted((_build_kwargs or {}).items())))
    if key not in _CACHE:
        _CACHE[key] = build_module(dtype=_dtype, **(_build_kwargs or {}))
    nc = _CACHE[key]
    in_maps = make_in_maps(outputs, feats, w_proj, b_proj, dtype=_dtype)
    res = run_bass_kernel_spmd(
        nc,
        in_maps,
        core_ids=list(range(N_CORES)),
        trace=_trace,
        **(_trace_kwargs or {}),
    )
    # each core returns out.T [K, E]; transpose back to [E, K] and stack
    out = np.stack([np.asarray(r["out"]).T for r in res.results])
    if _trace:
        _CACHE["last_results"] = res
    return out


# revision 7
# speedup vs baseline: 1.1589x; 1.0130x over previous
"""Trainium2 Bass kernel for nn_Encoder segment-reduce.

Reference computation (per sample b):
    cls = onehot(argmax_k outputs[b])            # [K, HW]
    sizes = cls.sum(HW) + 0.01                   # [K]
    feat_set = feats[b] @ cls.T / sizes          # [F, K]
    out[b] = w_proj @ feat_set + bias            # [E, K]

Kernel strategy (pure data parallel: 1 sample per NeuronCore, 8 cores).

Segment-reduce FIRST (the cheap contraction), projection second:
    feat_setT[k, f] = sum_hw onehot[hw, k] * featsT[hw, f]
with the onehot chunk [128hw, 21] as the PE's stationary operand and featsT
chunks [128hw, 512f] as the moving operand.  The four f-group matmuls of each
hw chunk are packed into the four 32-column groups of the PE array via
tile_position=(0, 32j): the stationary onehot only occupies 21 of 128 array
columns, so the four matmuls execute concurrently (measured 4ns stagger) and
the stream is purely DMA-paced.  One [128, 512] PSUM tile holds all four
accumulators (f-group j at partitions 32j..32j+21).

feats dtype is fp8 e3m4 (TRN FP8_EXP3): N(0,1) data fits the e3m4 range and
its 4 mantissa bits give rel err ~1.2e-2 end to end (threshold 2e-2), halving
HBM traffic vs bf16: 8.39 MB feats + 1 MB wT (bf16) + 0.34 MB outputs (f32)
~= 9.75 MB/core -- the kernel is DMA-bound at the per-core HBM limit.  The
matmuls run in normal (single-rate) fp8 mode: double-fp8 would upcast
operands to e6m3 and destroy e3m4's 4th mantissa bit (measured 3.0e-2).

DMA order: outputs (f32) + bias first so the DVE argmax fills the initial
feats window, then the feats blocks (2MB sustained-rate blocks, tapering to
single-chunk pieces at the end so the stream tail is not gated on a full
transfer), then wT (only needed by the projection tail).  The per-chunk
size-count matmul is interleaved with the stream so the PE never serializes
behind the (DVE-paced) argmax.

Tail (all PE stages packed into array tile groups):
  - a dummy-matmul burst bridges the 1/sizes window so the HAM clock gate
    keeps the PE at 2.4 GHz through the tail (it otherwise drops to 1.2 GHz
    after the low-duty DMA-paced stream);
  - 1/sizes is replicated to all four column groups with one bf16 matmul,
    then fused into the PSUM->SBUF copies (DVE + ACT halves);
  - the 16 [21,128]->[128,21] transposes run 4-at-a-time (one per 32-row
    row group, via a replicated identity);
  - the 16 projection matmuls (feat_set chunk stationary, wT moving) run
    4-at-a-time in the four column groups, giving four partial [21, 256]
    sums at partitions 32j; one final matmul against the replicated identity
    combines them, with the bias pre-loaded into its PSUM accumulator via a
    [1,21]x[1,256] outer product.
The output is stored as [K, E]; the host transposes when gathering.

dtype: "fp8" (e3m4 feats, rel err ~1.2e-2) or "bf16" (rel err ~3e-3).
"""

import numpy as np

import concourse.bacc as bacc
import concourse.bass as bass
import concourse.mybir as mybir
import concourse.tile as tile
from concourse.bass import ds, ts
from concourse.bass_utils import run_bass_kernel_spmd
from concourse.masks import make_identity

# Problem shapes (hardcoded per contract)
B = 8
K = 21
H = 64
W = 64
HW = H * W            # 4096
F = 2048
E = 256
P = 128
FC = F // P           # 16 f-chunks of 128
FG = 4                # f-groups of 512 (PE column groups)
FGW = F // FG         # 512
N_T = HW // P         # 32 hw chunks
N_CORES = 8

F32 = mybir.dt.float32
BF16 = mybir.dt.bfloat16
FP8 = mybir.dt.float8e3   # e3m4: 4 mantissa bits

DTYPE = "fp8"         # "fp8" or "bf16"


def build_module(dtype=DTYPE, warmup=100, endwarm=36):
    mm_dt = FP8 if dtype == "fp8" else BF16
    nc = bacc.Bacc("TRN2", target_bir_lowering=False, debug=False)

    # outputs host-transposed to [p, t, k] (pixel-major).
    outputs_d = nc.dram_tensor("outputs_in", [P, N_T, K], F32, kind="ExternalInput")
    # featsT host-permuted to [p, t, fgrp, fj]: featsT[t*128+p, fgrp*512+fj].
    feats_d = nc.dram_tensor(
        "feats_in", [P, N_T, FG, FGW], mm_dt, kind="ExternalInput"
    )
    wT_d = nc.dram_tensor("wT_in", [F, E], BF16, kind="ExternalInput")
    bias_d = nc.dram_tensor("bias_in", [E], F32, kind="ExternalInput")
    # out.T -- the host transposes each sample's [K, E] result when gathering.
    out_d = nc.dram_tensor("out", [K, E], F32, kind="ExternalOutput")

    # feats DMA blocks (start chunk, n chunks): a 1MB starter so the stream
    # begins early, 2MB sustained-rate blocks, then a taper so the last
    # chunks are not gated on a full 2MB transfer.
    blocks = [(0, 4), (4, 8), (12, 8), (20, 8), (28, 2), (30, 1), (31, 1)]

    with tile.TileContext(nc) as tc:
        with (
            tc.tile_pool(name="consts", bufs=1) as consts,
            tc.tile_pool(name="feats", bufs=len(blocks)) as feats_pool,
            tc.tile_pool(name="small", bufs=4) as small,
            tc.tile_pool(name="outp", bufs=1) as outp,
            tc.tile_pool(name="ps_fs", bufs=1, space="PSUM") as ps_fs,
            tc.tile_pool(name="ps_sz", bufs=1, space="PSUM") as ps_sz,
            tc.tile_pool(name="ps_tr", bufs=1, space="PSUM") as ps_tr,
            tc.tile_pool(name="ps_misc", bufs=1, space="PSUM") as ps_misc,
        ):
            # Bulk DMAs in FIFO order on the sync HWDGE queue.
            outputs_sb = consts.tile([P, N_T, K], F32)
            nc.sync.dma_start(out=outputs_sb, in_=outputs_d.ap())
            bias_sb = consts.tile([1, E], F32)
            nc.sync.dma_start(
                out=bias_sb, in_=bias_d.ap().rearrange("(o e) -> o e", o=1)
            )
            feats_r = feats_d.ap()
            fgs = []
            for bi, (t0, tb) in enumerate(blocks):
                fg = feats_pool.tile([P, tb, FG, FGW], mm_dt, name=f"fg{bi}",
                                     tag="fg")
                nc.sync.dma_start(out=fg, in_=feats_r[:, ds(t0, tb)])
                fgs.append(fg)
            wT_sb = consts.tile([P, FC, E], BF16)
            nc.sync.dma_start(
                out=wT_sb, in_=wT_d.ap().rearrange("(fc p) e -> p fc e", p=P)
            )

            # PE warm-up: HAM holds the PE at 1.2 GHz until ~3.4us of
            # sustained activity; dummy matmuls bridge the initial DMA wait.
            warm_w = consts.tile([P, 64], BF16)
            nc.vector.memset(warm_w, 0.0)
            warm_ps = ps_misc.tile([P, 64], F32, tag="warm")
            for _ in range(warmup):
                nc.tensor.matmul(warm_ps[0:64, :], lhsT=warm_w, rhs=warm_w)

            # Constants.  rep_sb[k, 32j+k'] = delta(k,k') replicates a [21]
            # partition vector to all four 32-row column-group offsets; it is
            # built with free-dim-shifted copies of the identity (partition
            # shifts are impossible on DVE, free shifts are not).
            ident = consts.tile([P, P], F32)
            make_identity(nc, ident)
            rep_sb = consts.tile([K, P], BF16)
            nc.vector.memset(rep_sb, 0.0)
            for j in range(FG):
                nc.vector.tensor_copy(rep_sb[:, ds(32 * j, K)], ident[:K, :K])
            ident21_b = consts.tile([K, K], BF16)
            nc.vector.tensor_copy(ident21_b, ident[:K, :K])
            # ident_rep[32j+k, k'] = delta(k,k'): per-column-group identity
            # for the tail transposes and the partial-sum combine, built on
            # the PE (rep_sb.T @ I21).
            identrep_ps = ps_tr.tile([P, K], F32, tag="t0")
            nc.tensor.matmul(
                identrep_ps, lhsT=rep_sb, rhs=ident21_b,
                start=True, stop=True,
            )
            ident_rep = consts.tile([P, K], BF16)
            nc.vector.tensor_copy(ident_rep, identrep_ps)
            ones_b = consts.tile([P, 2], mm_dt)
            nc.vector.memset(ones_b, 1.0)
            ones21_b = consts.tile([1, K], BF16)
            nc.vector.memset(ones21_b, 1.0)
            bias_b = consts.tile([1, E], BF16)
            nc.vector.tensor_copy(bias_b, bias_sb)
            # Force the ACT engine's table load off the critical path.
            dummy_act = small.tile([1, 2], F32, tag="da")
            nc.scalar.activation(
                out=dummy_act, in_=bias_sb[:, 0:2],
                func=mybir.ActivationFunctionType.Copy,
            )

            # Phase 1 (DVE only): onehot = (outT == rowmax) per hw chunk.
            oh_all = consts.tile([P, N_T, K], mm_dt)
            for t in range(N_T):
                rowmax = small.tile([P, 1], F32)
                nc.vector.tensor_reduce(
                    rowmax, outputs_sb[:, t, :], mybir.AxisListType.X,
                    mybir.AluOpType.max,
                )
                nc.vector.tensor_scalar(
                    out=oh_all[:, t, :],
                    in0=outputs_sb[:, t, :],
                    scalar1=rowmax,
                    scalar2=None,
                    op0=mybir.AluOpType.is_equal,
                )

            # Segment-reduce stream.  Per hw chunk: one size-count matmul
            # (onehot.T @ ones -> [21, 2]) plus four f-group matmuls packed
            # into the four PE column groups, accumulating [128, 512] PSUM
            # (f-group j at partitions 32j..32j+21) across all 32 chunks.
            fs_ps = ps_fs.tile([P, FGW], F32)
            sz_ps = ps_sz.tile([K, 2], F32, tag="sz")
            for bi, (t0, tb) in enumerate(blocks):
                fg = fgs[bi]
                for ti in range(tb):
                    t = t0 + ti
                    oh_t = oh_all[:, t, :]
                    nc.tensor.matmul(
                        sz_ps,
                        lhsT=oh_t,
                        rhs=ones_b,
                        start=(t == 0),
                        stop=(t == N_T - 1),
                    )
                    for j in range(FG):
                        nc.tensor.matmul(
                            fs_ps[ds(32 * j, K), :],
                            lhsT=oh_t,
                            rhs=fg[:, ti, j, :],
                            start=(t == 0),
                            stop=(t == N_T - 1),
                            tile_position=(0, 32 * j),
                        )

            # Keep the PE clock gate warm through the 1/sizes window so the
            # tail runs at 2.4 GHz.
            for _ in range(endwarm):
                nc.tensor.matmul(warm_ps[0:64, :], lhsT=warm_w, rhs=warm_w)

            # 1/sizes, replicated to all four column groups on the PE.
            sizes_sb = small.tile([K, 1], F32, tag="sizes")
            nc.vector.tensor_scalar_add(sizes_sb, sz_ps[:, 0:1], 0.01)
            recip = small.tile([K, 1], F32, tag="recip")
            nc.vector.reciprocal(recip, sizes_sb)
            recip_b = small.tile([K, 1], BF16, tag="recipb")
            nc.vector.tensor_copy(recip_b, recip)
            recip_ps = ps_sz.tile([P, 1], F32, name="recip128", tag="sz")
            nc.tensor.matmul(recip_ps, lhsT=rep_sb, rhs=recip_b,
                             start=True, stop=True)
            recip128 = small.tile([P, 1], F32, tag="r128")
            nc.vector.tensor_copy(recip128, recip_ps)

            # Scale by 1/sizes during the PSUM->SBUF copy (DVE + ACT halves).
            fs_sc = consts.tile([P, FGW], BF16)
            nc.vector.tensor_scalar_mul(
                fs_sc[:, 0 : FGW // 2], fs_ps[:, 0 : FGW // 2], recip128
            )
            nc.scalar.activation(
                out=fs_sc[:, ds(FGW // 2, FGW // 2)],
                in_=fs_ps[:, ds(FGW // 2, FGW // 2)],
                func=mybir.ActivationFunctionType.Copy,
                scale=recip128,
            )

            # Transposes, 4 concurrent per round (one per 32-row row group).
            # f-chunk fc = 4j + c lives in column group j at free cols
            # 128c..128c+128 of fs_sc.
            fsT_sb = consts.tile([P, FC, K], BF16)
            for c in range(4):
                trps = []
                for j in range(FG):
                    fc = 4 * j + c
                    trp = ps_tr.tile([P, K], BF16, name=f"trp{fc}",
                                     tag=f"t{j}")
                    nc.tensor.transpose(
                        trp,
                        fs_sc[ds(32 * j, K), ts(c, P)],
                        ident_rep[ds(32 * j, K), :],
                        tile_position=(32 * j, 0),
                    )
                    trps.append((fc, trp))
                for i, (fc, trp) in enumerate(trps):
                    if i % 2 == 0:
                        nc.vector.tensor_copy(fsT_sb[:, fc, :], trp)
                    else:
                        nc.scalar.activation(
                            out=fsT_sb[:, fc, :], in_=trp,
                            func=mybir.ActivationFunctionType.Copy,
                        )

            # Projection, 4 concurrent per round: column group j accumulates
            # the partial sum over its four f-chunks at partitions 32j.
            proj_ps = ps_misc.tile([P, E], F32, tag="warm", name="proj")
            for r in range(4):
                for j in range(FG):
                    fc = 4 * j + r
                    nc.tensor.matmul(
                        proj_ps[ds(32 * j, K), :],
                        lhsT=fsT_sb[:, fc, :],
                        rhs=wT_sb[:, fc, :],
                        start=(r == 0),
                        stop=(r == 3),
                        tile_position=(0, 32 * j),
                    )
            proj_sb = consts.tile([P, E], BF16)
            nc.vector.tensor_copy(proj_sb, proj_ps)

            # Combine the four partials + bias: final[k, e] =
            # sum_j proj[32j+k, e] + bias[e].
            final_ps = ps_misc.tile([K, E], F32, tag="warm", name="final")
            nc.tensor.matmul(final_ps, lhsT=ones21_b, rhs=bias_b,
                             start=True, stop=False, skip_group_check=True)
            nc.tensor.matmul(final_ps, lhsT=ident_rep, rhs=proj_sb,
                             start=False, stop=True, skip_group_check=True)
            out_sb = outp.tile([K, E], F32)
            nc.vector.tensor_copy(out_sb, final_ps)
            nc.sync.dma_start(out=out_d.ap(), in_=out_sb)

    nc.compile()
    return nc


_CACHE = {}


def make_in_maps(outputs, feats, w_proj, b_proj, dtype=DTYPE):
    import ml_dtypes

    mm_np = ml_dtypes.float8_e3m4 if dtype == "fp8" else ml_dtypes.bfloat16
    outputs = np.asarray(outputs, dtype=np.float32)
    # [B, K, H, W] -> per sample [p, t, k] (pixel-major: hw = t*128 + p)
    outputs_t = np.ascontiguousarray(
        outputs.reshape(B, K, N_T, P).transpose(0, 3, 2, 1)
    )
    feats = np.asarray(feats, dtype=np.float32).astype(mm_np)
    # [B, F, H, W] -> per sample [p, t, fgrp, fj] = featsT[t*128+p, fgrp*512+fj]
    feats_sh = np.ascontiguousarray(
        feats.reshape(B, FG, FGW, N_T, P).transpose(0, 4, 3, 1, 2)
    )
    wT = np.ascontiguousarray(
        np.asarray(w_proj, dtype=np.float32).T.astype(ml_dtypes.bfloat16)
    )
    bias = np.ascontiguousarray(np.asarray(b_proj, dtype=np.float32))
    return [
        {
            "outputs_in": outputs_t[b],
            "feats_in": feats_sh[b],
            "wT_in": wT,
            "bias_in": bias,
        }
        for b in range(B)
    ]


def kernel(outputs, feats, w_proj, b_proj, _trace=False, _trace_kwargs=None,
           _dtype=DTYPE, _build_kwargs=None):
    key = (_dtype, tuple(sorted((_build_kwargs or {}).items())))
    if key not in _CACHE:
        _CACHE[key] = build_module(dtype=_dtype, **(_build_kwargs or {}))
    nc = _CACHE[key]
    in_maps = make_in_maps(outputs, feats, w_proj, b_proj, dtype=_dtype)
    res = run_bass_kernel_spmd(
        nc,
        in_maps,
        core_ids=list(range(N_CORES)),
        trace=_trace,
        **(_trace_kwargs or {}),
    )
    # each core returns out.T [K, E]; transpose back to [E, K] and stack
    out = np.stack([np.asarray(r["out"]).T for r in res.results])
    if _trace:
        _CACHE["last_results"] = res
    return out
